# revision 6
# baseline (speedup 1.0000x reference)
"""ColorNorm Trainium2 kernel (v2: fp16 HBM I/O).

Problem: per-sample 3x3 color-matching solve over N=1024*1024 pixels.
  A = src[b] (3,N), B = dst[b] (3,N)
  AAt = Ac@Ac.T + 1e-3 I ; BAt = Bc@Ac.T ; x = BAt@inv(AAt)
  out[b] = x@Ac + Bmean
Sharding: data-parallel over batch (16 samples -> 8 cores x 2 samples).

v2 layout: HBM tensors are fp16 (host casts fp32->fp16 on the way in and
fp16->fp32 on the way out; rel-err budget 2e-2 dwarfs fp16 rounding).
All loads/stores are straight fp16 DMAs issued from the SP engine (HWDGE).

Per-core pipeline (fp16 data plane, fp32 accumulation/solve):
  pass1: DVE tensor_mul (fp16 2x) computes most of the 12 cross products
         per sample; a few run on GpSimd. Reductions are split between
         DVE tensor_scalar+accum (4x) and PE ones-matmuls into PSUM
         [1,512] quadrant slots. ScalarE Square+accum does the 3 A-diags.
         Raw channel sums ride on DVE ts+accum.
  solve: 3x3 inverse via adjugate (tiny fp32 DVE ops on partition 0).
  pass2: out_i = sum_j x_ij*A_j + d_i via fp16 PE matmuls with diag(x_ij)
         stationary weights accumulating in fp32 PSUM; ScalarE evicts with
         the +d_i bias fused, writing fp16; fp16 stores.
"""

import os
import sys

for _p in ("/opt/trn_rl_repo", "/opt/pypackages"):
    if _p not in sys.path:
        sys.path.append(_p)

from contextlib import ExitStack

import numpy as np

import concourse.bacc as bacc
import concourse.bass as bass
import concourse.tile as tile
from concourse import bass_isa, masks, mybir
from concourse._compat import with_exitstack

# ---- hardcoded problem geometry (per core) ----
B_CORE = 2          # samples per core
C = 3               # channels
H = W = 1024
N = H * W           # 1048576 pixels per channel
P = 128             # SBUF partitions
F = N // P          # 8192 free elems per partition per channel
HB = 4096           # B half-channel free size
MM = 512            # matmul free-dim chunk (one PSUM bank)
NCORES = 8
RIDGE = 1e-3

# ---- work placement knobs ----
GP_N = int(os.environ.get("CN_GP_N", "3"))        # BA-pair muls on GpSimd/sample
PE_N = int(os.environ.get("CN_PE_N", "9"))        # BA pairs reduced on PE/sample
PE_AA = int(os.environ.get("CN_PE_AA", "0"))      # AA pairs reduced on PE/sample
RAW_A = os.environ.get("CN_RAW_A", "dve")         # dve | pe
RAW_B = os.environ.get("CN_RAW_B", "dve")         # dve | pe
PTW = int(os.environ.get("CN_PTW", "512"))        # pass2 psum tile width
PS_BUFS = int(os.environ.get("CN_PS_BUFS", "3"))
B_BUFS = int(os.environ.get("CN_B_BUFS", "3"))
A_BUFS = int(os.environ.get("CN_A_BUFS", "6"))
SCRB_BUFS = int(os.environ.get("CN_SCRB", "3"))
GSCR_BUFS = int(os.environ.get("CN_GSCR", "1"))
DVE_TAIL = int(os.environ.get("CN_DVE_TAIL", "1"))  # last-sample ch0 on DVE
ST_ENG = os.environ.get("CN_ST_ENG", "sync")      # store DMA engine
LD_ENG = os.environ.get("CN_LD_ENG", "sync")      # load DMA engine

F32 = mybir.dt.float32
F16 = mybir.dt.float16
ALU = mybir.AluOpType
ACTF = mybir.ActivationFunctionType

A_CROSS = [(0, 1), (0, 2), (1, 2)]


def _rd(ap, dims):
    """Rebuild an AP keeping its partition dim, replacing free dims."""
    return bass.AP(ap.tensor, ap.offset, [ap.ap[0]] + dims)


@with_exitstack
def _colornorm(ctx: ExitStack, tc: "tile.TileContext", src, dst, out):
    nc = tc.nc
    srcv = src.rearrange("b c (p q) w -> b c p (q w)", p=P)  # [2,3,128,8192]
    dstv = dst.rearrange("b c (p q) w -> b c p (q w)", p=P)
    outv = out.rearrange("b c (p q) w -> b c p (q w)", p=P)

    ld_eng = getattr(nc, LD_ENG)
    st_eng = getattr(nc, ST_ENG)

    singles = ctx.enter_context(tc.tile_pool(name="singles", bufs=1))
    a_pool = ctx.enter_context(tc.tile_pool(name="a_pool", bufs=A_BUFS))
    b_pool = ctx.enter_context(tc.tile_pool(name="b_pool", bufs=B_BUFS))
    scr_pool = ctx.enter_context(tc.tile_pool(name="scr", bufs=1))
    scrb_pool = ctx.enter_context(tc.tile_pool(name="scrb", bufs=SCRB_BUFS))
    gscr_pool = ctx.enter_context(tc.tile_pool(name="gscr", bufs=GSCR_BUFS))
    acc_pool = ctx.enter_context(tc.tile_pool(name="accs", bufs=2))
    solve_pool = ctx.enter_context(tc.tile_pool(name="solve", bufs=2))
    dg_pool = ctx.enter_context(tc.tile_pool(name="dg", bufs=1))
    stage_pool = ctx.enter_context(tc.tile_pool(name="stage", bufs=2))
    ps_stat = ctx.enter_context(tc.tile_pool(name="ps_stat", bufs=1, space="PSUM"))
    ps_red = ctx.enter_context(tc.tile_pool(name="ps_red", bufs=4, space="PSUM"))
    ps_out = ctx.enter_context(tc.tile_pool(name="ps_out", bufs=PS_BUFS, space="PSUM"))

    ones = singles.tile([P, 1], F32)
    nc.vector.memset(ones, 1.0)
    ones16 = singles.tile([P, 1], F16)
    nc.vector.memset(ones16, 1.0)
    eye = singles.tile([P, P], F16)
    masks.make_identity(nc, eye[:])

    # acc column layout (per sample):
    #  0-2   AA cross (DVE-reduced)       3-5  A diag (fold target)
    #  6-23  BA halves (DVE): 6+2m+h      24-29 A raw halves/full
    #  30-35 B raw halves
    NCOL = 36

    for s in range(B_CORE):
        # ------------- loads (straight fp16) -------------
        a_t = [a_pool.tile([P, F], F16, tag="ach", name="ach") for _ in range(C)]
        for c in range(C):
            ld_eng.dma_start(out=a_t[c][:], in_=srcv[s, c])
        b_t = [[None, None] for _ in range(C)]
        for c in range(C):
            for h in range(2):
                b_t[c][h] = b_pool.tile([P, HB], F16, tag="bh", name="bh")
                ld_eng.dma_start(out=b_t[c][h][:],
                                 in_=dstv[s, c][:, h * HB:(h + 1) * HB])

        acc = acc_pool.tile([P, NCOL], F32, tag="acc", name="acc")
        nc.vector.memset(acc[:], 0.0)

        # ---- PE reduction slots: quantity q -> bank q//4, row 32*(q%4)
        # slots: 0..PE_N-1 BA pairs; then AA pairs (PE_AA); then raw A; raw B
        n_slots = PE_N + PE_AA + 3 * (RAW_A == "pe") + 3 * (RAW_B == "pe")
        assert n_slots <= 16
        n_banks = (n_slots + 3) // 4
        psa = [ps_red.tile([P, MM], F32, tag="psa", name="psa")
               for _ in range(n_banks)]
        slot_mm_seen = [0] * n_slots
        slot_mm_total = [16] * n_slots

        def pe_reduce(slot, x_ap, width, off):
            """ones-matmul partition-reduce of x_ap (width cols at free
            offset off within the channel) into psum slot."""
            bank, grp = slot // 4, slot % 4
            for m in range(width // MM):
                first = slot_mm_seen[slot] == 0
                slot_mm_seen[slot] += 1
                last = slot_mm_seen[slot] == slot_mm_total[slot]
                nc.tensor.matmul(
                    psa[bank][32 * grp:32 * grp + 1, :], ones16[:],
                    x_ap[:, m * MM:(m + 1) * MM],
                    start=first, stop=last,
                    tile_position=(0, 32 * grp))

        # one [P,F] DVE scratch per sample: products are reduced in place
        # (ts identity rewrite), raw sums dump their mandatory out here too.
        dscr = scr_pool.tile([P, F], F16, tag="scr", name="dscr")

        def dve_reduce(col, x_ap, width, dump=None):
            nc.vector.tensor_scalar(
                out=dump if dump is not None else x_ap,
                in0=x_ap, scalar1=1.0, scalar2=0.0,
                op0=ALU.mult, op1=ALU.add, accum_out=acc[:, col:col + 1])

        # ---- A-channel stats ----
        asq = acc_pool.tile([P, 6], F32, tag="asq", name="asq")
        ascr = scr_pool.tile([P, HB], F16, tag="ascr", name="ascr")
        for c in range(C):
            if RAW_A == "pe":
                pe_reduce(PE_N + PE_AA + c, a_t[c][:], F, 0)
            else:
                dve_reduce(24 + 2 * c, a_t[c][:], F, dump=dscr[:])
            for h in range(2):
                nc.scalar.activation(
                    out=ascr[:], in_=a_t[c][:, h * HB:(h + 1) * HB],
                    func=ACTF.Square,
                    accum_out=asq[:, 2 * c + h: 2 * c + h + 1])
        nc.vector.reduce_sum(out=acc[:, 3:6].rearrange("p (c o) -> p c o", o=1),
                             in_=asq[:, 0:6].rearrange("p (c h) -> p c h", h=2),
                             axis=mybir.AxisListType.X)

        # ---- AA pairs (full-channel muls on DVE) ----
        for k, (i, j) in enumerate(A_CROSS):
            if k < PE_AA:
                scr = scrb_pool.tile([P, F], F16, tag="scrf", name="scrf",
                                     bufs=1)
                nc.vector.tensor_mul(out=scr[:], in0=a_t[i][:], in1=a_t[j][:])
                pe_reduce(PE_N + k, scr[:], F, 0)
            else:
                nc.vector.tensor_mul(out=dscr[:], in0=a_t[i][:], in1=a_t[j][:])
                dve_reduce(k, dscr[:], F)

        # ---- B stats (half-channel granularity) ----
        for c in range(C):
            for h in range(2):
                if RAW_B == "pe":
                    pe_reduce(PE_N + PE_AA + 3 * (RAW_A == "pe") + c,
                              b_t[c][h][:], HB, h * HB)
                else:
                    dve_reduce(30 + 2 * c + h, b_t[c][h][:], HB,
                               dump=dscr[:, 0:HB])
                for j in range(C):
                    m = 3 * c + j
                    on_gp = m >= 9 - GP_N
                    if on_gp:
                        scr_ap = gscr_pool.tile([P, HB], F16, tag="gscr",
                                                name="gscr")[:]
                        nc.gpsimd.tensor_mul(out=scr_ap, in0=b_t[c][h][:],
                                             in1=a_t[j][:, h * HB:(h + 1) * HB])
                    elif m < PE_N:
                        scr_ap = scrb_pool.tile([P, HB], F16, tag="scrb",
                                                name="scrb")[:]
                        nc.vector.tensor_mul(out=scr_ap, in0=b_t[c][h][:],
                                             in1=a_t[j][:, h * HB:(h + 1) * HB])
                    else:
                        scr_ap = dscr[:, 0:HB]
                        nc.vector.tensor_mul(out=scr_ap, in0=b_t[c][h][:],
                                             in1=a_t[j][:, h * HB:(h + 1) * HB])
                    if m < PE_N:
                        pe_reduce(m, scr_ap, HB, h * HB)
                    else:
                        nc.vector.tensor_scalar(
                            out=scr_ap, in0=scr_ap,
                            scalar1=1.0, scalar2=0.0,
                            op0=ALU.mult, op1=ALU.add,
                            accum_out=acc[:, 6 + 2 * m + h: 7 + 2 * m + h])

        # ---- cross-partition reduce on PE: ones.T @ acc -> [1, NCOL] ----
        pst = ps_stat.tile([1, 40], F32, tag="pst", name="pst")
        nc.tensor.matmul(pst[0:1, 0:NCOL], ones[:], acc[:],
                         start=True, stop=True)
        stats = solve_pool.tile([1, 40], F32, tag="stats", name="stats")
        nc.vector.tensor_copy(out=stats[0:1, 0:NCOL], in_=pst[0:1, 0:NCOL])

        # ---- PE-slot partials -> prow -> all-reduce ----
        BA9 = solve_pool.tile([1, 9], F32, tag="BA9", name="BA9")
        if n_slots:
            prow = solve_pool.tile([P, 16], F32, tag="prow", name="prow")
            nc.vector.memset(prow[:], 0.0)
            for q in range(n_slots):
                bank, grp = q // 4, q % 4
                nc.vector.reduce_sum(
                    out=prow[32 * grp:32 * grp + 1, q:q + 1],
                    in_=psa[bank][32 * grp:32 * grp + 1, :],
                    axis=mybir.AxisListType.X)
            prow2 = solve_pool.tile([P, 16], F32, tag="prow2", name="prow2")
            nc.gpsimd.partition_all_reduce(
                prow2[:], prow[:], channels=P,
                reduce_op=bass_isa.ReduceOp.add)

        # assemble BA9 [1,9]
        if PE_N == 9:
            nc.vector.tensor_copy(out=BA9[:], in_=prow2[0:1, 0:9])
        else:
            if PE_N:
                nc.vector.tensor_copy(out=BA9[0:1, 0:PE_N],
                                      in_=prow2[0:1, 0:PE_N])
            if PE_N < 9:
                nc.vector.reduce_sum(
                    out=BA9[0:1, PE_N:9], axis=mybir.AxisListType.X,
                    in_=stats[0:1, 6 + 2 * PE_N:24].rearrange(
                        "p (k h) -> p k h", h=2))

        # AA cross sums -> cr3 [1,3]
        cr3 = solve_pool.tile([1, 3], F32, tag="cr3", name="cr3")
        if PE_AA:
            nc.vector.tensor_copy(out=cr3[0:1, 0:PE_AA],
                                  in_=prow2[0:1, PE_N:PE_N + PE_AA])
        if PE_AA < 3:
            nc.vector.tensor_copy(out=cr3[0:1, PE_AA:3],
                                  in_=stats[0:1, PE_AA:3])

        # raw sums
        sumA = solve_pool.tile([1, 3], F32, tag="sumA", name="sumA")
        sumB = solve_pool.tile([1, 3], F32, tag="sumB", name="sumB")
        if RAW_A == "pe":
            nc.vector.tensor_copy(
                out=sumA[:], in_=prow2[0:1, PE_N + PE_AA:PE_N + PE_AA + 3])
        else:
            nc.vector.reduce_sum(out=sumA[:], axis=mybir.AxisListType.X,
                                 in_=stats[0:1, 24:30].rearrange(
                                     "p (c h) -> p c h", h=2))
        if RAW_B == "pe":
            o = PE_N + PE_AA + 3 * (RAW_A == "pe")
            nc.vector.tensor_copy(out=sumB[:], in_=prow2[0:1, o:o + 3])
        else:
            nc.vector.reduce_sum(out=sumB[:], axis=mybir.AxisListType.X,
                                 in_=stats[0:1, 30:36].rearrange(
                                     "p (c h) -> p c h", h=2))

        # ---------------- 3x3 solve on partition 0 ----------------
        Am = solve_pool.tile([1, 3], F32, tag="Am", name="Am")
        Bm = solve_pool.tile([1, 3], F32, tag="Bm", name="Bm")
        nc.vector.tensor_scalar_mul(out=Am[:], in0=sumA[:], scalar1=1.0 / N)
        nc.vector.tensor_scalar_mul(out=Bm[:], in0=sumB[:], scalar1=1.0 / N)

        AA9 = solve_pool.tile([1, 9], F32, tag="AA9", name="AA9")
        # diag: AA9[0,4,8] = acc[3,4,5]
        nc.vector.tensor_copy(out=_rd(AA9[0:1, 0:1], [[4, 3]]),
                              in_=stats[0:1, 3:6])
        # off-diag pairs: (1,3)<-c01, (2,6)<-c02, (5,7)<-c12
        nc.vector.tensor_copy(out=_rd(AA9[0:1, 1:2], [[2, 2]]),
                              in_=_rd(cr3[0:1, 0:1], [[0, 2]]))
        nc.vector.tensor_copy(out=_rd(AA9[0:1, 2:3], [[4, 2]]),
                              in_=_rd(cr3[0:1, 1:2], [[0, 2]]))
        nc.vector.tensor_copy(out=_rd(AA9[0:1, 5:6], [[2, 2]]),
                              in_=_rd(cr3[0:1, 2:3], [[0, 2]]))

        # centered: AAc = AA - N*Am Am^T (+ridge); BAc = BA - N*Bm Am^T
        outer = solve_pool.tile([1, 9], F32, tag="outer", name="outer")
        o3x3 = outer[0:1, :].rearrange("p (i j) -> p i j", j=3)
        nc.vector.tensor_mul(out=o3x3, in0=_rd(Am[0:1, 0:1], [[1, 3], [0, 3]]),
                             in1=_rd(Am[0:1, 0:1], [[0, 3], [1, 3]]))
        AAc = solve_pool.tile([1, 9], F32, tag="AAc", name="AAc")
        nc.vector.scalar_tensor_tensor(out=AAc[:], in0=outer[:],
                                       scalar=-float(N), in1=AA9[:],
                                       op0=ALU.mult, op1=ALU.add)
        dg_ap = _rd(AAc[0:1, 0:1], [[4, 3]])
        nc.vector.tensor_scalar_add(out=dg_ap, in0=dg_ap, scalar1=RIDGE)
        nc.vector.tensor_mul(out=o3x3, in0=_rd(Bm[0:1, 0:1], [[1, 3], [0, 3]]),
                             in1=_rd(Am[0:1, 0:1], [[0, 3], [1, 3]]))
        BAc = solve_pool.tile([1, 9], F32, tag="BAc", name="BAc")
        nc.vector.scalar_tensor_tensor(out=BAc[:], in0=outer[:],
                                       scalar=-float(N), in1=BA9[:],
                                       op0=ALU.mult, op1=ALU.add)

        # inverse via adjugate: M2 = 6x6 tiling of AAc (mod-3 access)
        M2 = solve_pool.tile([1, 36], F32, tag="M2", name="M2")
        for dr in (0, 3):
            for dc in (0, 3):
                nc.vector.tensor_copy(
                    out=_rd(M2[0:1, 6 * dr + dc: 6 * dr + dc + 1],
                            [[6, 3], [1, 3]]),
                    in_=AAc[0:1, :].rearrange("p (i j) -> p i j", j=3))
        t1 = solve_pool.tile([1, 9], F32, tag="t1", name="t1")
        t2 = solve_pool.tile([1, 9], F32, tag="t2", name="t2")
        nc.vector.tensor_mul(out=t1[0:1, :].rearrange("p (i j) -> p i j", j=3),
                             in0=_rd(M2[0:1, 7:8], [[6, 3], [1, 3]]),
                             in1=_rd(M2[0:1, 14:15], [[6, 3], [1, 3]]))
        nc.vector.tensor_mul(out=t2[0:1, :].rearrange("p (i j) -> p i j", j=3),
                             in0=_rd(M2[0:1, 8:9], [[6, 3], [1, 3]]),
                             in1=_rd(M2[0:1, 13:14], [[6, 3], [1, 3]]))
        cof = solve_pool.tile([1, 9], F32, tag="cof", name="cof")
        nc.vector.tensor_sub(out=cof[:], in0=t1[:], in1=t2[:])

        det = solve_pool.tile([1, 1], F32, tag="det", name="det")
        dscr = solve_pool.tile([1, 3], F32, tag="dscr", name="dscr")
        nc.vector.scalar_tensor_tensor(
            out=dscr[:], in0=AAc[0:1, 0:3], scalar=1.0, in1=cof[0:1, 0:3],
            op0=ALU.mult, op1=ALU.mult, accum_out=det[:])
        rdet = solve_pool.tile([1, 1], F32, tag="rdet", name="rdet")
        nc.vector.reciprocal(out=rdet[:], in_=det[:])

        inv9 = solve_pool.tile([1, 9], F32, tag="inv9", name="inv9")
        nc.vector.tensor_scalar_mul(
            out=inv9[0:1, :].rearrange("p (i j) -> p i j", j=3),
            in0=_rd(cof[0:1, 0:1], [[1, 3], [3, 3]]),  # cof^T
            scalar1=rdet[:])

        # x = BAc @ inv  (tmp27[i,k,j] = BAc[i,j]*inv[j,k], reduce j)
        tmp27 = solve_pool.tile([1, 27], F32, tag="tmp27", name="tmp27")
        nc.vector.tensor_mul(
            out=tmp27[0:1, :].rearrange("p (i k j) -> p i k j", k=3, j=3),
            in0=_rd(BAc[0:1, 0:1], [[3, 3], [0, 3], [1, 3]]),
            in1=_rd(inv9[0:1, 0:1], [[0, 3], [1, 3], [3, 3]]))
        x9 = solve_pool.tile([1, 9], F32, tag="x9", name="x9")
        nc.vector.reduce_sum(
            out=x9[0:1, :].rearrange("p (i k) -> p i k", k=3),
            in_=tmp27[0:1, :].rearrange("p (i k j) -> p i k j", k=3, j=3),
            axis=mybir.AxisListType.X)

        # d = Bm - x@Am
        tmp9 = solve_pool.tile([1, 9], F32, tag="tmp9", name="tmp9")
        nc.vector.tensor_mul(
            out=tmp9[0:1, :].rearrange("p (i j) -> p i j", j=3),
            in0=x9[0:1, :].rearrange("p (i j) -> p i j", j=3),
            in1=_rd(Am[0:1, 0:1], [[0, 3], [1, 3]]))
        xAm = solve_pool.tile([1, 3], F32, tag="xAm", name="xAm")
        nc.vector.reduce_sum(out=xAm[:], axis=mybir.AxisListType.X,
                             in_=tmp9[0:1, :].rearrange("p (i j) -> p i j", j=3))
        sol = solve_pool.tile([1, 12], F32, tag="sol", name="sol")
        nc.vector.tensor_copy(out=sol[0:1, 0:9], in_=x9[:])
        nc.vector.tensor_sub(out=sol[0:1, 9:12], in0=Bm[:], in1=xAm[:])

        # broadcast x,d to all partitions
        xb = solve_pool.tile([P, 12], F32, tag="xb", name="xb")
        nc.gpsimd.partition_broadcast(xb[:], sol[0:1, 0:12])

        # diag(x_ij) fp16 weight tiles
        dg = [[dg_pool.tile([P, P], F16, tag=f"dg{i}{j}", name=f"dg{i}{j}")
               for j in range(C)] for i in range(C)]
        for i in range(C):
            for j in range(C):
                nc.vector.tensor_scalar_mul(
                    out=dg[i][j][:], in0=eye[:],
                    scalar1=xb[:, 3 * i + j: 3 * i + j + 1])

        # -------- pass 2: out_i = sum_j x_ij A_j + d_i --------
        QW = 2048  # store-quarter width
        NQ = F // QW
        dve_ch = {0} if (DVE_TAIL and s == B_CORE - 1) else set()
        for g in range(NQ):
            for i in range(C):
                stage = stage_pool.tile([P, QW], F16, tag="stage", name="stage")
                ga = slice(g * QW, (g + 1) * QW)
                if i in dve_ch:
                    t0 = scrb_pool.tile([P, HB], F16, tag="scrb",
                                        name="scrb")[:, 0:QW]
                    t1_ = scrb_pool.tile([P, HB], F16, tag="scrb",
                                         name="scrb")[:, 0:QW]
                    nc.vector.tensor_scalar(
                        out=t0, in0=a_t[0][:, ga],
                        scalar1=xb[:, 3 * i: 3 * i + 1],
                        scalar2=xb[:, 9 + i: 10 + i],
                        op0=ALU.mult, op1=ALU.add)
                    nc.vector.tensor_scalar(
                        out=t1_, in0=a_t[1][:, ga],
                        scalar1=xb[:, 3 * i + 1: 3 * i + 2], scalar2=0.0,
                        op0=ALU.mult, op1=ALU.add)
                    nc.vector.tensor_add(out=t0, in0=t0, in1=t1_)
                    nc.vector.tensor_scalar(
                        out=t1_, in0=a_t[2][:, ga],
                        scalar1=xb[:, 3 * i + 2: 3 * i + 3], scalar2=0.0,
                        op0=ALU.mult, op1=ALU.add)
                    nc.vector.tensor_add(out=stage[:], in0=t0, in1=t1_)
                    st_eng.dma_start(out=outv[s, i][:, ga], in_=stage[:])
                    continue
                for hh in range(QW // PTW):
                    pt = ps_out.tile([P, PTW], F32, tag="pt", name="pt")
                    for j in range(C):
                        for cc in range(PTW // MM):
                            o0 = hh * PTW + cc * MM
                            nc.tensor.matmul(
                                pt[:, cc * MM:(cc + 1) * MM], dg[i][j][:],
                                a_t[j][:, g * QW + o0: g * QW + o0 + MM],
                                start=(j == 0), stop=(j == 2))
                    nc.scalar.add(out=stage[:, hh * PTW:(hh + 1) * PTW],
                                  in_=pt[:], add=xb[:, 9 + i: 10 + i])
                st_eng.dma_start(out=outv[s, i][:, ga], in_=stage[:])


def build_nc() -> "bass.Bass":
    nc = bacc.Bacc("TRN2", target_bir_lowering=False)
    src = nc.dram_tensor("src", [B_CORE, C, H, W], F16, kind="ExternalInput")
    dst = nc.dram_tensor("dst", [B_CORE, C, H, W], F16, kind="ExternalInput")
    out = nc.dram_tensor("out", [B_CORE, C, H, W], F16, kind="ExternalOutput")
    with tile.TileContext(nc) as tc:
        _colornorm(tc, src[:], dst[:], out[:])
    nc.finalize()
    return nc


_NC = None


def _get_nc():
    global _NC
    if _NC is None:
        _NC = build_nc()
    return _NC


TRACE = False
LAST_RESULT = None  # BassKernelResults of the most recent run (for profiling)


def kernel(src, dst):
    from concourse.bass_utils import run_bass_kernel_spmd

    global LAST_RESULT
    src = np.asarray(src, dtype=np.float32).astype(np.float16)
    dst = np.asarray(dst, dtype=np.float32).astype(np.float16)
    assert src.shape == (NCORES * B_CORE, C, H, W), src.shape
    nc = _get_nc()
    in_maps = [
        {
            "src": np.ascontiguousarray(src[i * B_CORE:(i + 1) * B_CORE]),
            "dst": np.ascontiguousarray(dst[i * B_CORE:(i + 1) * B_CORE]),
        }
        for i in range(NCORES)
    ]
    res = run_bass_kernel_spmd(nc, in_maps, core_ids=list(range(NCORES)),
                               trace=TRACE)
    LAST_RESULT = res
    return np.concatenate([r["out"] for r in res.results],
                          axis=0).astype(np.float32)


# revision 8
# speedup vs baseline: 1.2907x; 1.2907x over previous
"""ColorNorm Trainium2 kernel (v3: fp16 HBM I/O, software-pipelined).

Problem: per-sample 3x3 color-matching solve over N=1024*1024 pixels.
  A = src[b] (3,N), B = dst[b] (3,N)
  AAt = Ac@Ac.T + 1e-3 I ; BAt = Bc@Ac.T ; x = BAt@inv(AAt)
  out[b] = x@Ac + Bmean
Sharding: data-parallel over batch (16 samples -> 8 cores x 2 samples).

HBM tensors are fp16 (host casts fp32<->fp16; rel-err budget 2e-2 dwarfs
fp16 rounding). Loads are SP-issued HWDGE DMAs; stores issue from the
Activation engine right after its PSUM eviction (keeps the SP queue free
of head-of-line blocking).

Engine placement per sample (fp16 data plane, fp32 accumulation/solve):
  pass1: DVE tensor_mul (2x) for most of the 12 cross products, a few on
         GpSimd. BA-pair reductions run as PE ones-matmuls into PSUM
         [1,512] quadrant slots; AA reductions + raw sums are DVE
         tensor_scalar+accum (4x, in-place rewrite of the product).
         ScalarE Square+accum does the 3 A-diags.
  solve: 3x3 inverse via adjugate (tiny fp32 DVE ops on partition 0).
  pass2: out_i = sum_j x_ij*A_j + d_i via fp16 PE matmuls with diag(x_ij)
         stationary weights; ScalarE evicts PSUM with the +d_i bias fused,
         writing fp16 stage tiles; fp16 stores.

Emission is software-pipelined across the two samples: sample 1's pass-1
B-groups interleave with sample 0's pass-2 output groups so the in-order
engine queues (PE especially) never drain between samples.
"""

import os
import sys

for _p in ("/opt/trn_rl_repo", "/opt/pypackages"):
    if _p not in sys.path:
        sys.path.append(_p)

from contextlib import ExitStack

import numpy as np

import concourse.bacc as bacc
import concourse.bass as bass
import concourse.tile as tile
from concourse import bass_isa, masks, mybir
from concourse._compat import with_exitstack

# ---- hardcoded problem geometry (per core) ----
B_CORE = 2          # samples per core
C = 3               # channels
H = W = 1024
N = H * W           # 1048576 pixels per channel
P = 128             # SBUF partitions
F = N // P          # 8192 free elems per partition per channel
HB = 4096           # B half-channel free size
MM = 512            # matmul free-dim chunk (one PSUM bank)
QW = 2048           # pass2 store-quarter width
NCORES = 8
RIDGE = 1e-3

# ---- work placement knobs ----
GP_N = int(os.environ.get("CN_GP_N", "3"))        # BA-pair muls on GpSimd/sample
PE_N = int(os.environ.get("CN_PE_N", "9"))        # BA pairs reduced on PE/sample
PTW = int(os.environ.get("CN_PTW", "512"))        # pass2 psum tile width
PS_BUFS = int(os.environ.get("CN_PS_BUFS", "3"))
B_BUFS = int(os.environ.get("CN_B_BUFS", "3"))
A_BUFS = int(os.environ.get("CN_A_BUFS", "6"))
SCRB_BUFS = int(os.environ.get("CN_SCRB", "3"))
GSCR_BUFS = int(os.environ.get("CN_GSCR", "1"))
DVE_TAIL = int(os.environ.get("CN_DVE_TAIL", "1"))  # last-sample ch0 on DVE
ST_ENG = os.environ.get("CN_ST_ENG", "scalar")    # store DMA engine
LD_ENG = os.environ.get("CN_LD_ENG", "sync")      # load DMA engine
ILV = int(os.environ.get("CN_ILV", "1"))          # interleave s1 pass1/s0 pass2

F32 = mybir.dt.float32
F16 = mybir.dt.float16
ALU = mybir.AluOpType
ACTF = mybir.ActivationFunctionType

A_CROSS = [(0, 1), (0, 2), (1, 2)]
NCOL = 36


def _rd(ap, dims):
    """Rebuild an AP keeping its partition dim, replacing free dims."""
    return bass.AP(ap.tensor, ap.offset, [ap.ap[0]] + dims)


@with_exitstack
def _colornorm(ctx: ExitStack, tc: "tile.TileContext", src, dst, out):
    nc = tc.nc
    srcv = src.rearrange("b c (p q) w -> b c p (q w)", p=P)  # [2,3,128,8192]
    dstv = dst.rearrange("b c (p q) w -> b c p (q w)", p=P)
    outv = out.rearrange("b c (p q) w -> b c p (q w)", p=P)

    ld_eng = getattr(nc, LD_ENG)
    st_eng = getattr(nc, ST_ENG)

    singles = ctx.enter_context(tc.tile_pool(name="singles", bufs=1))
    a_pool = ctx.enter_context(tc.tile_pool(name="a_pool", bufs=A_BUFS))
    b_pool = ctx.enter_context(tc.tile_pool(name="b_pool", bufs=B_BUFS))
    scr_pool = ctx.enter_context(tc.tile_pool(name="scr", bufs=1))
    scrb_pool = ctx.enter_context(tc.tile_pool(name="scrb", bufs=SCRB_BUFS))
    gscr_pool = ctx.enter_context(tc.tile_pool(name="gscr", bufs=GSCR_BUFS))
    acc_pool = ctx.enter_context(tc.tile_pool(name="accs", bufs=2))
    solve_pool = ctx.enter_context(tc.tile_pool(name="solve", bufs=2))
    dg_pool = ctx.enter_context(tc.tile_pool(name="dg", bufs=2))
    stage_pool = ctx.enter_context(tc.tile_pool(name="stage", bufs=2))
    ps_stat = ctx.enter_context(tc.tile_pool(name="ps_stat", bufs=1, space="PSUM"))
    ps_red = ctx.enter_context(tc.tile_pool(name="ps_red", bufs=4, space="PSUM"))
    ps_out = ctx.enter_context(tc.tile_pool(name="ps_out", bufs=PS_BUFS, space="PSUM"))

    ones = singles.tile([P, 1], F32)
    nc.vector.memset(ones, 1.0)
    ones16 = singles.tile([P, 1], F16)
    nc.vector.memset(ones16, 1.0)
    eye = singles.tile([P, P], F16)
    masks.make_identity(nc, eye[:])

    def emit_loads(st):
        s = st["s"]
        st["a_t"] = [a_pool.tile([P, F], F16, tag="ach", name="ach")
                     for _ in range(C)]
        for c in range(C):
            ld_eng.dma_start(out=st["a_t"][c][:], in_=srcv[s, c])
        st["b_t"] = [[None, None] for _ in range(C)]
        for c in range(C):
            for h in range(2):
                t = b_pool.tile([P, HB], F16, tag="bh", name="bh")
                st["b_t"][c][h] = t
                ld_eng.dma_start(out=t[:],
                                 in_=dstv[s, c][:, h * HB:(h + 1) * HB])

    def pe_reduce(st, slot, x_ap, width):
        """ones-matmul partition-reduce into psum slot (bank, 32*grp row)."""
        bank, grp = slot // 4, slot % 4
        psa, seen, total = st["psa"], st["slot_seen"], st["slot_total"]
        for m in range(width // MM):
            first = seen[slot] == 0
            seen[slot] += 1
            nc.tensor.matmul(
                psa[bank][32 * grp:32 * grp + 1, :], ones16[:],
                x_ap[:, m * MM:(m + 1) * MM],
                start=first, stop=seen[slot] == total[slot],
                tile_position=(0, 32 * grp))

    def dve_reduce(st, col, x_ap, dump=None):
        acc = st["acc"]
        nc.vector.tensor_scalar(
            out=dump if dump is not None else x_ap,
            in0=x_ap, scalar1=1.0, scalar2=0.0,
            op0=ALU.mult, op1=ALU.add, accum_out=acc[:, col:col + 1])

    def emit_pass1_a(st):
        """acc/psum setup + A raw sums, squares, AA pairs."""
        a_t = st["a_t"]
        st["acc"] = acc_pool.tile([P, NCOL], F32, tag="acc", name="acc")
        nc.vector.memset(st["acc"][:], 0.0)
        n_slots = PE_N
        n_banks = max(1, (n_slots + 3) // 4)
        st["psa"] = [ps_red.tile([P, MM], F32, tag="psa", name="psa")
                     for _ in range(n_banks)]
        st["slot_seen"] = [0] * 16
        st["slot_total"] = [16] * 16
        st["dscr"] = scr_pool.tile([P, F], F16, tag="scr", name="dscr")
        st["ascr"] = scr_pool.tile([P, HB], F16, tag="ascr", name="ascr")
        st["asq"] = acc_pool.tile([P, 6], F32, tag="asq", name="asq")
        acc, asq, ascr, dscr = st["acc"], st["asq"], st["ascr"], st["dscr"]
        for c in range(C):
            dve_reduce(st, 24 + 2 * c, a_t[c][:], dump=dscr[:])
            for h in range(2):
                nc.scalar.activation(
                    out=ascr[:], in_=a_t[c][:, h * HB:(h + 1) * HB],
                    func=ACTF.Square,
                    accum_out=asq[:, 2 * c + h: 2 * c + h + 1])
        nc.vector.reduce_sum(out=acc[:, 3:6].rearrange("p (c o) -> p c o", o=1),
                             in_=asq[:, 0:6].rearrange("p (c h) -> p c h", h=2),
                             axis=mybir.AxisListType.X)
        for k, (i, j) in enumerate(A_CROSS):
            nc.vector.tensor_mul(out=dscr[:], in0=a_t[i][:], in1=a_t[j][:])
            dve_reduce(st, k, dscr[:])

    def emit_b_group(st, c, h):
        """raw B sum + the 3 BA pairs for dst channel c, half h."""
        a_t, b_t, acc, dscr = st["a_t"], st["b_t"], st["acc"], st["dscr"]
        dve_reduce(st, 30 + 2 * c + h, b_t[c][h][:], dump=dscr[:, 0:HB])
        for j in range(C):
            m = 3 * c + j
            # spread GpSimd pairs across the (c,h) groups: j==2 column
            on_gp = (m % 3 == 2) and (m // 3 < GP_N)
            if on_gp:
                scr_ap = gscr_pool.tile([P, HB], F16, tag="gscr",
                                        name="gscr")[:]
                nc.gpsimd.tensor_mul(out=scr_ap, in0=b_t[c][h][:],
                                     in1=a_t[j][:, h * HB:(h + 1) * HB])
            elif m < PE_N:
                scr_ap = scrb_pool.tile([P, HB], F16, tag="scrb",
                                        name="scrb")[:]
                nc.vector.tensor_mul(out=scr_ap, in0=b_t[c][h][:],
                                     in1=a_t[j][:, h * HB:(h + 1) * HB])
            else:
                scr_ap = dscr[:, 0:HB]
                nc.vector.tensor_mul(out=scr_ap, in0=b_t[c][h][:],
                                     in1=a_t[j][:, h * HB:(h + 1) * HB])
            if m < PE_N:
                pe_reduce(st, m, scr_ap, HB)
            else:
                nc.vector.tensor_scalar(
                    out=scr_ap, in0=scr_ap, scalar1=1.0, scalar2=0.0,
                    op0=ALU.mult, op1=ALU.add,
                    accum_out=acc[:, 6 + 2 * m + h: 7 + 2 * m + h])

    def emit_finalize_solve(st):
        acc, psa = st["acc"], st["psa"]
        # cross-partition reduce on PE: ones.T @ acc -> [1, NCOL]
        pst = ps_stat.tile([1, 40], F32, tag="pst", name="pst")
        nc.tensor.matmul(pst[0:1, 0:NCOL], ones[:], acc[:],
                         start=True, stop=True)
        stats = solve_pool.tile([1, 40], F32, tag="stats", name="stats")
        nc.vector.tensor_copy(out=stats[0:1, 0:NCOL], in_=pst[0:1, 0:NCOL])

        BA9 = solve_pool.tile([1, 9], F32, tag="BA9", name="BA9")
        if PE_N:
            prow = solve_pool.tile([P, 16], F32, tag="prow", name="prow")
            nc.vector.memset(prow[:], 0.0)
            for q in range(PE_N):
                bank, grp = q // 4, q % 4
                nc.vector.reduce_sum(
                    out=prow[32 * grp:32 * grp + 1, q:q + 1],
                    in_=psa[bank][32 * grp:32 * grp + 1, :],
                    axis=mybir.AxisListType.X)
            prow2 = solve_pool.tile([P, 16], F32, tag="prow2", name="prow2")
            nc.gpsimd.partition_all_reduce(
                prow2[:], prow[:], channels=P,
                reduce_op=bass_isa.ReduceOp.add)
        if PE_N == 9:
            nc.vector.tensor_copy(out=BA9[:], in_=prow2[0:1, 0:9])
        else:
            if PE_N:
                nc.vector.tensor_copy(out=BA9[0:1, 0:PE_N],
                                      in_=prow2[0:1, 0:PE_N])
            nc.vector.reduce_sum(
                out=BA9[0:1, PE_N:9], axis=mybir.AxisListType.X,
                in_=stats[0:1, 6 + 2 * PE_N:24].rearrange(
                    "p (k h) -> p k h", h=2))

        sumA = solve_pool.tile([1, 3], F32, tag="sumA", name="sumA")
        sumB = solve_pool.tile([1, 3], F32, tag="sumB", name="sumB")
        nc.vector.reduce_sum(out=sumA[:], axis=mybir.AxisListType.X,
                             in_=stats[0:1, 24:30].rearrange(
                                 "p (c h) -> p c h", h=2))
        nc.vector.reduce_sum(out=sumB[:], axis=mybir.AxisListType.X,
                             in_=stats[0:1, 30:36].rearrange(
                                 "p (c h) -> p c h", h=2))

        # ---------------- 3x3 solve on partition 0 ----------------
        Am = solve_pool.tile([1, 3], F32, tag="Am", name="Am")
        Bm = solve_pool.tile([1, 3], F32, tag="Bm", name="Bm")
        nc.vector.tensor_scalar_mul(out=Am[:], in0=sumA[:], scalar1=1.0 / N)
        nc.vector.tensor_scalar_mul(out=Bm[:], in0=sumB[:], scalar1=1.0 / N)

        AA9 = solve_pool.tile([1, 9], F32, tag="AA9", name="AA9")
        nc.vector.tensor_copy(out=_rd(AA9[0:1, 0:1], [[4, 3]]),
                              in_=stats[0:1, 3:6])
        nc.vector.tensor_copy(out=_rd(AA9[0:1, 1:2], [[2, 2]]),
                              in_=_rd(stats[0:1, 0:1], [[0, 2]]))
        nc.vector.tensor_copy(out=_rd(AA9[0:1, 2:3], [[4, 2]]),
                              in_=_rd(stats[0:1, 1:2], [[0, 2]]))
        nc.vector.tensor_copy(out=_rd(AA9[0:1, 5:6], [[2, 2]]),
                              in_=_rd(stats[0:1, 2:3], [[0, 2]]))

        outer = solve_pool.tile([1, 9], F32, tag="outer", name="outer")
        o3x3 = outer[0:1, :].rearrange("p (i j) -> p i j", j=3)
        nc.vector.tensor_mul(out=o3x3, in0=_rd(Am[0:1, 0:1], [[1, 3], [0, 3]]),
                             in1=_rd(Am[0:1, 0:1], [[0, 3], [1, 3]]))
        AAc = solve_pool.tile([1, 9], F32, tag="AAc", name="AAc")
        nc.vector.scalar_tensor_tensor(out=AAc[:], in0=outer[:],
                                       scalar=-float(N), in1=AA9[:],
                                       op0=ALU.mult, op1=ALU.add)
        dg_ap = _rd(AAc[0:1, 0:1], [[4, 3]])
        nc.vector.tensor_scalar_add(out=dg_ap, in0=dg_ap, scalar1=RIDGE)
        nc.vector.tensor_mul(out=o3x3, in0=_rd(Bm[0:1, 0:1], [[1, 3], [0, 3]]),
                             in1=_rd(Am[0:1, 0:1], [[0, 3], [1, 3]]))
        BAc = solve_pool.tile([1, 9], F32, tag="BAc", name="BAc")
        nc.vector.scalar_tensor_tensor(out=BAc[:], in0=outer[:],
                                       scalar=-float(N), in1=BA9[:],
                                       op0=ALU.mult, op1=ALU.add)

        M2 = solve_pool.tile([1, 36], F32, tag="M2", name="M2")
        for dr in (0, 3):
            for dc in (0, 3):
                nc.vector.tensor_copy(
                    out=_rd(M2[0:1, 6 * dr + dc: 6 * dr + dc + 1],
                            [[6, 3], [1, 3]]),
                    in_=AAc[0:1, :].rearrange("p (i j) -> p i j", j=3))
        t1 = solve_pool.tile([1, 9], F32, tag="t1", name="t1")
        t2 = solve_pool.tile([1, 9], F32, tag="t2", name="t2")
        nc.vector.tensor_mul(out=t1[0:1, :].rearrange("p (i j) -> p i j", j=3),
                             in0=_rd(M2[0:1, 7:8], [[6, 3], [1, 3]]),
                             in1=_rd(M2[0:1, 14:15], [[6, 3], [1, 3]]))
        nc.vector.tensor_mul(out=t2[0:1, :].rearrange("p (i j) -> p i j", j=3),
                             in0=_rd(M2[0:1, 8:9], [[6, 3], [1, 3]]),
                             in1=_rd(M2[0:1, 13:14], [[6, 3], [1, 3]]))
        cof = solve_pool.tile([1, 9], F32, tag="cof", name="cof")
        nc.vector.tensor_sub(out=cof[:], in0=t1[:], in1=t2[:])

        det = solve_pool.tile([1, 1], F32, tag="det", name="det")
        dscr2 = solve_pool.tile([1, 3], F32, tag="dscr2", name="dscr2")
        nc.vector.scalar_tensor_tensor(
            out=dscr2[:], in0=AAc[0:1, 0:3], scalar=1.0, in1=cof[0:1, 0:3],
            op0=ALU.mult, op1=ALU.mult, accum_out=det[:])
        rdet = solve_pool.tile([1, 1], F32, tag="rdet", name="rdet")
        nc.vector.reciprocal(out=rdet[:], in_=det[:])

        inv9 = solve_pool.tile([1, 9], F32, tag="inv9", name="inv9")
        nc.vector.tensor_scalar_mul(
            out=inv9[0:1, :].rearrange("p (i j) -> p i j", j=3),
            in0=_rd(cof[0:1, 0:1], [[1, 3], [3, 3]]),  # cof^T
            scalar1=rdet[:])

        tmp27 = solve_pool.tile([1, 27], F32, tag="tmp27", name="tmp27")
        nc.vector.tensor_mul(
            out=tmp27[0:1, :].rearrange("p (i k j) -> p i k j", k=3, j=3),
            in0=_rd(BAc[0:1, 0:1], [[3, 3], [0, 3], [1, 3]]),
            in1=_rd(inv9[0:1, 0:1], [[0, 3], [1, 3], [3, 3]]))
        x9 = solve_pool.tile([1, 9], F32, tag="x9", name="x9")
        nc.vector.reduce_sum(
            out=x9[0:1, :].rearrange("p (i k) -> p i k", k=3),
            in_=tmp27[0:1, :].rearrange("p (i k j) -> p i k j", k=3, j=3),
            axis=mybir.AxisListType.X)

        tmp9 = solve_pool.tile([1, 9], F32, tag="tmp9", name="tmp9")
        nc.vector.tensor_mul(
            out=tmp9[0:1, :].rearrange("p (i j) -> p i j", j=3),
            in0=x9[0:1, :].rearrange("p (i j) -> p i j", j=3),
            in1=_rd(Am[0:1, 0:1], [[0, 3], [1, 3]]))
        xAm = solve_pool.tile([1, 3], F32, tag="xAm", name="xAm")
        nc.vector.reduce_sum(out=xAm[:], axis=mybir.AxisListType.X,
                             in_=tmp9[0:1, :].rearrange("p (i j) -> p i j",
                                                        j=3))
        sol = solve_pool.tile([1, 12], F32, tag="sol", name="sol")
        nc.vector.tensor_copy(out=sol[0:1, 0:9], in_=x9[:])
        nc.vector.tensor_sub(out=sol[0:1, 9:12], in0=Bm[:], in1=xAm[:])

        xb = solve_pool.tile([P, 12], F32, tag="xb", name="xb")
        nc.gpsimd.partition_broadcast(xb[:], sol[0:1, 0:12])
        st["xb"] = xb

        dg = [[dg_pool.tile([P, P], F16, tag=f"dg{i}{j}", name=f"dg{i}{j}")
               for j in range(C)] for i in range(C)]
        for i in range(C):
            for j in range(C):
                nc.vector.tensor_scalar_mul(
                    out=dg[i][j][:], in0=eye[:],
                    scalar1=xb[:, 3 * i + j: 3 * i + j + 1])
        st["dg"] = dg

    def emit_pass2_group(st, g, i):
        """output channel i, quarter g: PE matmuls + evict + store."""
        s, a_t, xb, dg = st["s"], st["a_t"], st["xb"], st["dg"]
        stage = stage_pool.tile([P, QW], F16, tag="stage", name="stage")
        ga = slice(g * QW, (g + 1) * QW)
        if i in st["dve_ch"]:
            t0 = scrb_pool.tile([P, HB], F16, tag="scrb", name="scrb")[:, 0:QW]
            t1_ = scrb_pool.tile([P, HB], F16, tag="scrb", name="scrb")[:, 0:QW]
            nc.vector.tensor_scalar(
                out=t0, in0=a_t[0][:, ga],
                scalar1=xb[:, 3 * i: 3 * i + 1],
                scalar2=xb[:, 9 + i: 10 + i],
                op0=ALU.mult, op1=ALU.add)
            nc.vector.tensor_scalar(
                out=t1_, in0=a_t[1][:, ga],
                scalar1=xb[:, 3 * i + 1: 3 * i + 2], scalar2=0.0,
                op0=ALU.mult, op1=ALU.add)
            nc.vector.tensor_add(out=t0, in0=t0, in1=t1_)
            nc.vector.tensor_scalar(
                out=t1_, in0=a_t[2][:, ga],
                scalar1=xb[:, 3 * i + 2: 3 * i + 3], scalar2=0.0,
                op0=ALU.mult, op1=ALU.add)
            nc.vector.tensor_add(out=stage[:], in0=t0, in1=t1_)
            st_eng.dma_start(out=outv[s, i][:, ga], in_=stage[:])
            return
        for hh in range(QW // PTW):
            pt = ps_out.tile([P, PTW], F32, tag="pt", name="pt")
            for j in range(C):
                for cc in range(PTW // MM):
                    o0 = hh * PTW + cc * MM
                    nc.tensor.matmul(
                        pt[:, cc * MM:(cc + 1) * MM], dg[i][j][:],
                        a_t[j][:, g * QW + o0: g * QW + o0 + MM],
                        start=(j == 0), stop=(j == 2))
            nc.scalar.add(out=stage[:, hh * PTW:(hh + 1) * PTW],
                          in_=pt[:], add=xb[:, 9 + i: 10 + i])
        st_eng.dma_start(out=outv[s, i][:, ga], in_=stage[:])

    # ---------------- pipeline driver ----------------
    NQ = F // QW
    sts = [{"s": s, "dve_ch": set()} for s in range(B_CORE)]
    if DVE_TAIL:
        sts[B_CORE - 1]["dve_ch"] = {0}

    emit_loads(sts[0])
    emit_loads(sts[1])
    emit_pass1_a(sts[0])
    for c in range(C):
        for h in range(2):
            emit_b_group(sts[0], c, h)
    emit_finalize_solve(sts[0])
    emit_pass1_a(sts[1])
    p2_groups = [(g, i) for g in range(NQ) for i in range(C)]
    if ILV:
        bg = [(c, h) for c in range(C) for h in range(2)]
        gi = iter(p2_groups)
        for idx, (c, h) in enumerate(bg):
            emit_b_group(sts[1], c, h)
            emit_pass2_group(sts[0], *next(gi))
            emit_pass2_group(sts[0], *next(gi))
        for g, i in gi:
            emit_pass2_group(sts[0], g, i)
    else:
        for c in range(C):
            for h in range(2):
                emit_b_group(sts[1], c, h)
        for g, i in p2_groups:
            emit_pass2_group(sts[0], g, i)
    emit_finalize_solve(sts[1])
    for g, i in p2_groups:
        emit_pass2_group(sts[1], g, i)


def build_nc() -> "bass.Bass":
    nc = bacc.Bacc("TRN2", target_bir_lowering=False)
    src = nc.dram_tensor("src", [B_CORE, C, H, W], F16, kind="ExternalInput")
    dst = nc.dram_tensor("dst", [B_CORE, C, H, W], F16, kind="ExternalInput")
    out = nc.dram_tensor("out", [B_CORE, C, H, W], F16, kind="ExternalOutput")
    with tile.TileContext(nc) as tc:
        _colornorm(tc, src[:], dst[:], out[:])
    nc.finalize()
    return nc


_NC = None


def _get_nc():
    global _NC
    if _NC is None:
        _NC = build_nc()
    return _NC


TRACE = False
LAST_RESULT = None  # BassKernelResults of the most recent run (for profiling)


def kernel(src, dst):
    from concourse.bass_utils import run_bass_kernel_spmd

    global LAST_RESULT
    src = np.asarray(src, dtype=np.float32).astype(np.float16)
    dst = np.asarray(dst, dtype=np.float32).astype(np.float16)
    assert src.shape == (NCORES * B_CORE, C, H, W), src.shape
    nc = _get_nc()
    in_maps = [
        {
            "src": np.ascontiguousarray(src[i * B_CORE:(i + 1) * B_CORE]),
            "dst": np.ascontiguousarray(dst[i * B_CORE:(i + 1) * B_CORE]),
        }
        for i in range(NCORES)
    ]
    res = run_bass_kernel_spmd(nc, in_maps, core_ids=list(range(NCORES)),
                               trace=TRACE)
    LAST_RESULT = res
    return np.concatenate([r["out"] for r in res.results],
                          axis=0).astype(np.float32)
